# revision 7
# baseline (speedup 1.0000x reference)
"""Trainium2 Bass kernel for nn_DivEncoder (grouped MLP + ELU + L2 norm).

Math (per batch row n):
  xg = x.reshape(D, V); zeta = einsum('duv,dv->du', W1, xg) + b1
  y_d = b2_d + sum_u W2[d,u] * elu(zeta[d,u]);  out = y / max(||y||, eps)

Decomposition used on device (m = min(zeta,0), e = exp(m)):
  elu(zeta) = zeta - m + e - 1
  y = c0 + sum_v wlin[d,v] x[d,v] + sum_u W2 e - sum_u W2 m
  c0 = b2 + sum_u W2 b1 - sum_u W2 ;  wlin = sum_u W2[d,u] W1[d,u,:]

Sharding: batch rows across 8 cores (512 rows each); weights replicated.

Per-core dataflow, 64 chunks of 128 features (8 groups):
  - DMA x[:,128c:128c+128] in, GPSIMD split to bf16 hi/lo, DMA-xbar
    transpose to feature-major [128 feat, 512 batch] bf16 tiles.
  - L1: 3 bf16 matmuls per 2-group block (K=32 row strip) -> z PSUM.
  - m = min(z + b1, 0) on DVE (PSUM->SBUF, fp16 out).
  - e = exp(m) on ACT (fp16).
  - L2: fp16 matmuls (+W2 on e, -W2 on m, M=32 zero-padded lhsT) and bf16
    wlin matmuls (M=128 zero-padded) accumulate 16 chunks into one PSUM
    bank at slot partitions 32k + 2*(c%16) + i.
  - Evac bank (+c0) once per 16 chunks; PE-transpose to batch-major;
    row norm (sqrt + reciprocal + one Newton step); DMA out.
"""
import sys
sys.path.insert(0, "/opt/trn_rl_repo")

import numpy as np
import ml_dtypes

import concourse.bass as bass
import concourse.bacc as bacc
import concourse.mybir as mybir
import concourse.tile as tile
from concourse import bass_utils

F32 = mybir.dt.float32
F16 = mybir.dt.float16
BF16 = mybir.dt.bfloat16
AL = mybir.AluOpType
AF = mybir.ActivationFunctionType

N, H, D, U, V = 4096, 8192, 512, 64, 16
NCORE = 8
R = N // NCORE          # 512 batch rows per core
CH = H // 128           # 64 chunks
BG = 4                  # bank groups (16 chunks each)
EPS = 1e-12

_cache = {}


def _build(loop_reps=1):
    nc = bacc.Bacc("TRN2", target_bir_lowering=False, debug=False,
                   enable_asserts=False, num_devices=NCORE)
    ap = {}
    ap["x"] = nc.dram_tensor("x", [R, H], F32, kind="ExternalInput").ap()
    ap["w1hi"] = nc.dram_tensor("w1hi", [CH, 128, 128], BF16, kind="ExternalInput").ap()
    ap["w1lo"] = nc.dram_tensor("w1lo", [CH, 128, 128], BF16, kind="ExternalInput").ap()
    ap["wlhi"] = nc.dram_tensor("wlhi", [CH, 128, 128], BF16, kind="ExternalInput").ap()
    ap["wllo"] = nc.dram_tensor("wllo", [CH, 128, 128], BF16, kind="ExternalInput").ap()
    ap["w2e"] = nc.dram_tensor("w2e", [CH, 128, 128], F16, kind="ExternalInput").ap()
    ap["w2m"] = nc.dram_tensor("w2m", [CH, 128, 128], F16, kind="ExternalInput").ap()
    ap["b1c"] = nc.dram_tensor("b1c", [CH, 128, 4], F32, kind="ExternalInput").ap()
    ap["c0s"] = nc.dram_tensor("c0s", [BG, 128, 1], F32, kind="ExternalInput").ap()
    ap["ident"] = nc.dram_tensor("ident", [128, 128], F32, kind="ExternalInput").ap()
    y_out = nc.dram_tensor("y", [R, D], F32, kind="ExternalOutput").ap()

    with tile.TileContext(nc) as tc:
        _emit(nc, tc, ap, y_out)
    nc.compile()
    return nc


def _emit(nc, tc, ap, y_out):
    with (
        tc.tile_pool(name="wres", bufs=1) as wres,      # resident weights
        tc.tile_pool(name="xin", bufs=3) as xin,        # x chunk stream
        tc.tile_pool(name="xsp", bufs=3) as xsp,        # split bf16
        tc.tile_pool(name="xtr", bufs=3) as xtr,        # transposed bf16
        tc.tile_pool(name="me", bufs=2) as mepool,      # m/e fp16 (2-chunk)
        tc.tile_pool(name="yfm", bufs=1) as yfm,        # assembled y + yT
        tc.tile_pool(name="zps", bufs=3, space="PSUM") as zps,
        tc.tile_pool(name="yps", bufs=2, space="PSUM") as yps,
        tc.tile_pool(name="sml", bufs=1) as sml,        # small norm tiles
    ):
        # ---- resident weights
        t_w1hi, t_w1lo, t_wlhi, t_wllo = [], [], [], []
        t_w2e, t_w2m, t_b1 = [], [], []
        for c in range(CH):
            w1h = wres.tile([128, 128], BF16, tag=f"w1h{c}")
            nc.sync.dma_start(w1h[:], ap["w1hi"][c])
            t_w1hi.append(w1h)
            w1l = wres.tile([128, 128], BF16, tag=f"w1l{c}")
            nc.sync.dma_start(w1l[:], ap["w1lo"][c])
            t_w1lo.append(w1l)
            wlh = wres.tile([128, 128], BF16, tag=f"wlh{c}")
            nc.sync.dma_start(wlh[:], ap["wlhi"][c])
            t_wlhi.append(wlh)
            wll = wres.tile([128, 128], BF16, tag=f"wll{c}")
            nc.sync.dma_start(wll[:], ap["wllo"][c])
            t_wllo.append(wll)
            w2e = wres.tile([128, 128], F16, tag=f"w2e{c}")  # 4x[128,32] packed
            nc.sync.dma_start(w2e[:], ap["w2e"][c])
            t_w2e.append(w2e)
            w2m = wres.tile([128, 128], F16, tag=f"w2m{c}")
            nc.sync.dma_start(w2m[:], ap["w2m"][c])
            t_w2m.append(w2m)
            b1 = wres.tile([128, 4], F32, tag=f"b1{c}")
            nc.sync.dma_start(b1[:], ap["b1c"][c])
            t_b1.append(b1)
        t_c0 = []
        for b in range(BG):
            c0 = wres.tile([128, 1], F32, tag=f"c0{b}")
            nc.sync.dma_start(c0[:], ap["c0s"][b])
            t_c0.append(c0)
        t_id = wres.tile([128, 128], F32, tag="ident")
        nc.sync.dma_start(t_id[:], ap["ident"][:])

        x_ap = ap["x"]
        y_banks = {}
        t_yfm = [yfm.tile([128, 512], F32, tag=f"yfm{b}", name=f"yfm{b}")
                 for b in range(BG)]

        # software-pipelined main loop over 2-chunk groups
        pend = None  # (group chunks' em-mm closures)
        for g in range(CH // 2 + 1):
            if g < CH // 2:
                cpair = (2 * g, 2 * g + 1)
                m_t = mepool.tile([128, 4096], F16, tag="m")
                e_t = mepool.tile([128, 4096], F16, tag="e")
                for ci, c in enumerate(cpair):
                    b = c // 16
                    cp = c % 16
                    if c % 16 == 0:
                        y_banks[b] = yps.tile([128, 512], F32, tag="ybank",
                                              name=f"ybank{b}")
                    ybank = y_banks[b]

                    # --- load x chunk [512 rows, 128 cols] as [128, 4*128]
                    xt = xin.tile([128, 512], F32, tag="xc")
                    for j in range(4):
                        nc.sync.dma_start(
                            xt[:, 128 * j:128 * (j + 1)],
                            x_ap[128 * j:128 * (j + 1), 128 * c:128 * (c + 1)])
                    # --- split bf16 hi/lo on GPSIMD
                    xh = xsp.tile([128, 512], BF16, tag="xh")
                    nc.gpsimd.tensor_copy(xh[:], xt[:])
                    xl = xsp.tile([128, 512], BF16, tag="xl")
                    nc.gpsimd.tensor_tensor(xl[:], xt[:], xh[:], AL.subtract)
                    # --- transpose to feature-major
                    xhT = xtr.tile([128, 512], BF16, tag="xhT")
                    xlT = xtr.tile([128, 512], BF16, tag="xlT")
                    for j in range(4):
                        nc.sync.dma_start_transpose(
                            xhT[:, 128 * j:128 * (j + 1)], xh[:, 128 * j:128 * (j + 1)])
                        nc.sync.dma_start_transpose(
                            xlT[:, 128 * j:128 * (j + 1)], xl[:, 128 * j:128 * (j + 1)])

                    # --- L1: z = W1 x (2 strips per z tile; 2 tiles)
                    zAB = [zps.tile([128, 1024], F32, tag="z", name=f"z{c}_{h}")
                           for h in range(2)]
                    for k in range(4):
                        zt = zAB[k // 2]
                        zsl = zt[:, 512 * (k % 2):512 * (k % 2) + 512]
                        row = slice(32 * k, 32 * k + 32)
                        tp = (32 * k, 0)
                        nc.tensor.matmul(zsl, t_w1hi[c][row, :], xhT[row, :],
                                         start=True, stop=False, tile_position=tp,
                                         skip_group_check=True)
                        nc.tensor.matmul(zsl, t_w1hi[c][row, :], xlT[row, :],
                                         start=False, stop=False, tile_position=tp,
                                         skip_group_check=True)
                        nc.tensor.matmul(zsl, t_w1lo[c][row, :], xhT[row, :],
                                         start=False, stop=True, tile_position=tp,
                                         skip_group_check=True)
                    # --- wlin matmuls (M=128, zero-padded), first of bank zeroes
                    first = (cp == 0)
                    nc.tensor.matmul(ybank[:, :], t_wlhi[c][:, :], xhT[:, :],
                                     start=first, stop=False, skip_group_check=True)
                    nc.tensor.matmul(ybank[:, :], t_wlhi[c][:, :], xlT[:, :],
                                     start=False, stop=False, skip_group_check=True)
                    nc.tensor.matmul(ybank[:, :], t_wllo[c][:, :], xhT[:, :],
                                     start=False, stop=False, skip_group_check=True)
                    # --- m pass (DVE): m = min(z + b1, 0) -> fp16
                    for k in range(4):
                        zt = zAB[k // 2]
                        zsl = zt[:, 512 * (k % 2):512 * (k % 2) + 512]
                        msl = m_t[:, 2048 * ci + 512 * k: 2048 * ci + 512 * k + 512]
                        nc.vector.tensor_scalar(msl, zsl, t_b1[c][:, k:k + 1], 0.0,
                                                AL.add, AL.min)
                # --- e pass (ACT) over both chunks at once
                nc.scalar.activation(e_t[:], m_t[:], AF.Exp)

                def em_mms(cpair=cpair, m_t=m_t, e_t=e_t):
                    for ci, c in enumerate(cpair):
                        b = c // 16
                        ybank = y_banks[b]
                        last_chunk = (c % 16 == 15)
                        for k in range(4):
                            esl = e_t[:, 2048 * ci + 512 * k: 2048 * ci + 512 * k + 512]
                            msl = m_t[:, 2048 * ci + 512 * k: 2048 * ci + 512 * k + 512]
                            ysl = ybank[32 * k:32 * k + 32, :]
                            nc.tensor.matmul(ysl, t_w2e[c][:, 32 * k:32 * k + 32], esl,
                                             start=False, stop=False,
                                             tile_position=(0, 32 * k),
                                             skip_group_check=True)
                            nc.tensor.matmul(ysl, t_w2m[c][:, 32 * k:32 * k + 32], msl,
                                             start=False,
                                             stop=(last_chunk and k == 3),
                                             tile_position=(0, 32 * k),
                                             skip_group_check=True)
                        if last_chunk:
                            nc.vector.tensor_scalar(t_yfm[b][:], ybank[:],
                                                    t_c0[b][:, 0:1], None, AL.add)
                return_pend = em_mms
            else:
                return_pend = None
            if pend is not None:
                pend()
            pend = return_pend

        # ---- norm + output (batch-major)
        for j in range(4):
            yT = xin.tile([128, 512], F32, tag="xc")
            for b in range(BG):
                pT = zps.tile([128, 128], F32, tag="z")
                nc.tensor.transpose(pT[:], t_yfm[b][:, 128 * j:128 * (j + 1)], t_id[:])
                nc.vector.tensor_copy(yT[:, 128 * b:128 * (b + 1)], pT[:])
            sq = xin.tile([128, 512], F32, tag="xc")
            nc.scalar.activation(sq[:], yT[:], AF.Square)
            ss = sml.tile([128, 1], F32, tag=f"ss{j}")
            nc.vector.reduce_sum(ss[:], sq[:], axis=mybir.AxisListType.X)
            s = sml.tile([128, 1], F32, tag=f"s{j}")
            nc.scalar.activation(s[:], ss[:], AF.Sqrt)
            nc.vector.tensor_scalar(s[:], s[:], float(EPS), None, AL.max)
            r0 = sml.tile([128, 1], F32, tag=f"r0{j}")
            nc.vector.reciprocal(r0[:], s[:])
            # one Newton step for rsqrt: r1 = r0*(1.5 - 0.5*ss*r0^2)
            t1 = sml.tile([128, 1], F32, tag=f"t1{j}")
            nc.vector.tensor_tensor(t1[:], r0[:], r0[:], AL.mult)
            nc.vector.tensor_tensor(t1[:], t1[:], ss[:], AL.mult)
            nc.vector.tensor_scalar(t1[:], t1[:], -0.5, 1.5, AL.mult, AL.add)
            r1 = sml.tile([128, 1], F32, tag=f"r1{j}")
            nc.vector.tensor_tensor(r1[:], r0[:], t1[:], AL.mult)
            nc.vector.tensor_scalar(yT[:], yT[:], r1[:], None, AL.mult)
            nc.sync.dma_start(y_out[128 * j:128 * (j + 1), :], yT[:])


def _pack_host(W1, b1, W2, b2):
    bf = ml_dtypes.bfloat16
    W1 = W1.astype(np.float32)
    b1 = b1.astype(np.float32)
    W2 = W2.astype(np.float32)
    b2 = b2.astype(np.float32)

    wlin = np.einsum('du,duv->dv', W2.astype(np.float64),
                     W1.astype(np.float64)).astype(np.float32)
    c0 = b2 + (W2 * b1).sum(-1) - W2.sum(-1)

    W1h = W1.astype(bf)
    W1l = (W1 - W1h.astype(np.float32)).astype(bf)
    wlh = wlin.astype(bf)
    wll = (wlin - wlh.astype(np.float32)).astype(bf)
    W2f = W2.astype(np.float16)

    w1hi = np.zeros((CH, 128, 128), bf)
    w1lo = np.zeros((CH, 128, 128), bf)
    wlhi = np.zeros((CH, 128, 128), bf)
    wllo = np.zeros((CH, 128, 128), bf)
    w2e = np.zeros((CH, 128, 128), np.float16)
    b1c = np.zeros((CH, 128, 4), np.float32)
    c0s = np.zeros((BG, 128, 1), np.float32)

    for c in range(CH):
        cp = c % 16
        for k in range(4):
            g0 = 8 * c + 2 * k
            g1 = g0 + 1
            # L1 lhsT [K=32 rows at 32k.., M=128]: rows j<16 -> g0 v=j, cols u
            w1hi[c, 32 * k:32 * k + 16, 0:64] = W1h[g0].T          # [v,u]
            w1hi[c, 32 * k + 16:32 * k + 32, 64:128] = W1h[g1].T
            w1lo[c, 32 * k:32 * k + 16, 0:64] = W1l[g0].T
            w1lo[c, 32 * k + 16:32 * k + 32, 64:128] = W1l[g1].T
            # wlin lhsT [128, 128]: row 32k+j -> col 32k+2cp+i
            wlhi[c, 32 * k:32 * k + 16, 32 * k + 2 * cp] = wlh[g0]
            wlhi[c, 32 * k + 16:32 * k + 32, 32 * k + 2 * cp + 1] = wlh[g1]
            wllo[c, 32 * k:32 * k + 16, 32 * k + 2 * cp] = wll[g0]
            wllo[c, 32 * k + 16:32 * k + 32, 32 * k + 2 * cp + 1] = wll[g1]
            # L2 lhsT [128, 32]: rows u-layout, cols 2cp+i
            w2e[c, 0:64, 32 * k + 2 * cp] = W2f[g0]
            w2e[c, 64:128, 32 * k + 2 * cp + 1] = W2f[g1]
            b1c[c, 0:64, k] = b1[g0]
            b1c[c, 64:128, k] = b1[g1]
            b_ = c // 16
            c0s[b_, 32 * k + 2 * cp, 0] = c0[g0]
            c0s[b_, 32 * k + 2 * cp + 1, 0] = c0[g1]
    w2m = -w2e
    # permutation matrix: transpose output col j (= d-local) <- slot s
    ident = np.zeros((128, 128), dtype=np.float32)
    for cp in range(16):
        for k in range(4):
            for i in range(2):
                jcol = 8 * cp + 2 * k + i
                slot = 32 * k + 2 * cp + i
                ident[slot, jcol] = 1.0
    return {"w1hi": w1hi, "w1lo": w1lo, "wlhi": wlhi, "wllo": wllo,
            "w2e": w2e, "w2m": w2m, "b1c": b1c, "c0s": c0s, "ident": ident}


def kernel(x, W1, b1, W2, b2):
    x = np.ascontiguousarray(np.asarray(x, dtype=np.float32))
    packed = _pack_host(np.asarray(W1), np.asarray(b1),
                        np.asarray(W2), np.asarray(b2))
    if "nc" not in _cache:
        _cache["nc"] = _build()
    nc = _cache["nc"]
    in_maps = []
    for i in range(NCORE):
        m = dict(packed)
        m["x"] = x[i * R:(i + 1) * R]
        in_maps.append(m)
    res = bass_utils.run_bass_kernel_spmd(nc, in_maps, core_ids=list(range(NCORE)))
    out = np.concatenate([res.results[i]["y"] for i in range(NCORE)], axis=0)
    return out.astype(np.float32)
